# revision 4
# baseline (speedup 1.0000x reference)
"""GQA attention + RoPE kernel for Trainium2, sharded over 8 NeuronCores.

Sharding: core c handles (batch b = c // 4, kv-head h = c % 4) — one GQA
group (4 query heads + its kv head) per core.  Each core computes its
group's attention end-to-end plus the partial W_o product for its 256
input features; the host sums the 4 partials per batch (the "all-reduce
after W_o" step) and stacks batches.

Device-side layout is fully transposed ("T layout", feature dim on SBUF
partitions, sequence on the free dim):
  - Q^T/K^T projections feed the scores matmul directly (S^T = K @ Q^T),
    so softmax'd probabilities come out already transposed for the P@V
    matmul — no on-chip transposes of the [T, T] score matrix.
  - RoPE is applied in T layout using a sign-permutation matmul (PERM)
    plus elementwise muls with precomputed cos/sin tables.
  - The P@V matmul's stationary operand is [V | ones] (or [ones | V]),
    so each accumulation emits the attention output AND the softmax
    denominator broadcast across 64 partitions in one pass.
  - exp() is evaluated without max-subtraction: scores for this problem
    are bounded (|s| < 4), far from fp32/bf16 overflow.
Numerics: bf16 matmul operands with fp32 PSUM accumulation everywhere;
fp32 partial outputs.
"""

import numpy as np
import ml_dtypes

import concourse.bass as bass
import concourse.tile as tile
from concourse import bacc, mybir
from concourse.bass_utils import run_bass_kernel_spmd

BF16 = mybir.dt.bfloat16
F32 = mybir.dt.float32
NP_BF16 = ml_dtypes.bfloat16

D_MODEL = 1024
T = 2048
N_HEADS = 16
N_KV = 4
GROUPS = 4
D_K = 64
ROPE_BASE = 10000.0
N_CORES = 8

DC = D_MODEL // 128  # 8 contraction chunks for projections
TK = T // 128        # 16 key chunks
NQ = T // 512        # 4 query tiles of 512


def build_kernel_body(tc):
    nc = tc.nc

    qT = nc.dram_tensor("qT", [D_MODEL, T], BF16, kind="ExternalInput").ap()
    kT = nc.dram_tensor("kT", [D_MODEL, T], BF16, kind="ExternalInput").ap()
    vT = nc.dram_tensor("vT", [D_MODEL, T], BF16, kind="ExternalInput").ap()
    wq = nc.dram_tensor("wq", [D_MODEL, 256], BF16, kind="ExternalInput").ap()
    wk2 = nc.dram_tensor("wk2", [D_MODEL, 128], BF16, kind="ExternalInput").ap()
    wv = nc.dram_tensor("wv", [D_MODEL, 64], BF16, kind="ExternalInput").ap()
    wo = nc.dram_tensor("wo", [256, 1024], BF16, kind="ExternalInput").ap()
    cos2 = nc.dram_tensor("cos2", [128, T], BF16, kind="ExternalInput").ap()
    sin2 = nc.dram_tensor("sin2", [128, T], BF16, kind="ExternalInput").ap()
    perm = nc.dram_tensor("perm", [128, 128], BF16, kind="ExternalInput").ap()
    y = nc.dram_tensor("y", [T, 1024], F32, kind="ExternalOutput").ap()

    with (
        tc.tile_pool(name="const", bufs=1) as const,
        tc.tile_pool(name="tmp", bufs=3) as tmp,
        tc.tile_pool(name="ptp", bufs=3) as ptp,
        tc.tile_pool(name="norm", bufs=2) as normp,
    ):
        qT_sb = const.tile([128, DC * T], BF16, tag="qT_sb")
        kT_sb = const.tile([128, DC * T], BF16, tag="kT_sb")
        vT_sb = const.tile([128, DC * T], BF16, tag="vT_sb")
        wq_sb = const.tile([128, DC * 256], BF16, tag="wq_sb")
        wk2_sb = const.tile([128, DC * 128], BF16, tag="wk2_sb")
        wv_sb = const.tile([128, DC * 64], BF16, tag="wv_sb")
        wo_sb = const.tile([128, 2 * 1024], BF16, tag="wo_sb")
        cos_sb = const.tile([128, T], BF16, tag="cos_sb")
        sin_sb = const.tile([128, T], BF16, tag="sin_sb")
        perm_sb = const.tile([128, 128], BF16, tag="perm_sb")
        qrope = const.tile([128, 2 * T], BF16, tag="qrope")
        krope = const.tile([128, T], BF16, tag="krope")
        # [V | ones] and [ones | V] stationary tiles, one 128-col block per
        # key chunk.  The ones half makes the P@V matmul emit the softmax
        # denominator broadcast over 64 partitions: A-form puts O in rows
        # 0:64 / l in 64:128, B-form mirrors it so each head's normalized
        # output lands in its own half of the onorm chunk.
        vextA = const.tile([128, TK * 128], BF16, tag="vextA")
        vextB = const.tile([128, TK * 128], BF16, tag="vextB")
        onorm = const.tile([128, 2 * T], BF16, tag="onorm")

        for c in range(DC):
            nc.sync.dma_start(qT_sb[:, c * T:(c + 1) * T],
                              qT.rearrange("(c p) t -> c p t", p=128)[c])
            nc.sync.dma_start(kT_sb[:, c * T:(c + 1) * T],
                              kT.rearrange("(c p) t -> c p t", p=128)[c])
            nc.sync.dma_start(vT_sb[:, c * T:(c + 1) * T],
                              vT.rearrange("(c p) t -> c p t", p=128)[c])
        for w_sb, w_dram, wcols, nch in ((wq_sb, wq, 256, DC), (wk2_sb, wk2, 128, DC),
                                         (wv_sb, wv, 64, DC), (wo_sb, wo, 1024, 2)):
            w3 = w_dram.rearrange("(c p) n -> c p n", p=128)
            for c in range(nch):
                nc.sync.dma_start(w_sb[:, c * wcols:(c + 1) * wcols], w3[c])
        nc.sync.dma_start(cos_sb[:], cos2[:])
        nc.sync.dma_start(sin_sb[:], sin2[:])
        nc.sync.dma_start(perm_sb[:], perm[:])
        nc.gpsimd.memset(vextA[:], 1.0)
        nc.gpsimd.memset(vextB[:], 1.0)

        # ---- projections (V plain; Q/K2 with fused RoPE) ----
        with tc.tile_pool(name="pp", bufs=6, space="PSUM") as pp:
            for i in range(TK):
                psv = pp.tile([128, 512], F32, tag="pp")
                for c in range(DC):
                    nc.tensor.matmul(
                        psv[:, 0:64],
                        vT_sb[:, c * T + i * 128: c * T + (i + 1) * 128],
                        wv_sb[:, c * 64:(c + 1) * 64],
                        start=(c == 0), stop=(c == DC - 1))
                nc.vector.tensor_copy(vextA[:, i * 128: i * 128 + 64], psv[:, 0:64])
                nc.vector.tensor_copy(vextB[:, i * 128 + 64: (i + 1) * 128], psv[:, 0:64])

            def project_rope(dst, dst_off, w_sb, wcols, ch, src_sb, st):
                # dst[:, dst_off+st*512 ...] = rope(W_chunk^T @ srcT tile)
                ps = pp.tile([128, 512], F32, tag="pp")
                for c in range(DC):
                    nc.tensor.matmul(
                        ps[:],
                        w_sb[:, c * wcols + ch * 128: c * wcols + ch * 128 + 128],
                        src_sb[:, c * T + st * 512: c * T + st * 512 + 512],
                        start=(c == 0), stop=(c == DC - 1))
                xts = tmp.tile([128, 512], BF16, tag="xts")
                nc.vector.tensor_copy(xts[:], ps[:])
                psr = pp.tile([128, 512], F32, tag="pp")
                nc.tensor.matmul(psr[:], perm_sb[:], xts[:], start=True, stop=True)
                t1 = tmp.tile([128, 512], BF16, tag="t1")
                nc.vector.tensor_mul(t1[:], ps[:], cos_sb[:, st * 512:(st + 1) * 512])
                t2 = tmp.tile([128, 512], BF16, tag="t2")
                nc.vector.tensor_mul(t2[:], psr[:], sin_sb[:, st * 512:(st + 1) * 512])
                nc.vector.tensor_add(
                    dst[:, dst_off + st * 512: dst_off + st * 512 + 512], t1[:], t2[:])

            for ch in range(2):
                for st in range(NQ):
                    project_rope(qrope, ch * T, wq_sb, 256, ch, qT_sb, st)
            for st in range(NQ):
                project_rope(krope, 0, wk2_sb, 128, 0, kT_sb, st)

        # ---- attention + output projection ----
        with (
            tc.tile_pool(name="stp", bufs=2, space="PSUM") as stp,
            tc.tile_pool(name="accp", bufs=2, space="PSUM") as accp,
            tc.tile_pool(name="yp", bufs=2, space="PSUM") as yp,
        ):
            for p in range(2):       # head pair (heads 2p, 2p+1)
                for qt in range(NQ):  # 512 queries per sweep
                    q0 = p * T + qt * 512
                    accA = accp.tile([128, 512], F32, tag="acc")
                    accB = accp.tile([128, 512], F32, tag="acc")
                    for i in range(TK):
                        st_ps = stp.tile([128, 1024], F32, tag="st")
                        nc.tensor.matmul(
                            st_ps[:, 0:512],
                            krope[0:64, i * 128:(i + 1) * 128],
                            qrope[0:64, q0:q0 + 512],
                            start=True, stop=True, tile_position=(0, 0))
                        nc.tensor.matmul(
                            st_ps[:, 512:1024],
                            krope[64:128, i * 128:(i + 1) * 128],
                            qrope[64:128, q0:q0 + 512],
                            start=True, stop=True, tile_position=(64, 0))
                        pt = ptp.tile([128, 1024], BF16, tag="pt")
                        nc.scalar.activation(
                            pt[:], st_ps[:],
                            mybir.ActivationFunctionType.Exp, scale=0.125)
                        nc.tensor.matmul(
                            accA[:], vextA[:, i * 128:(i + 1) * 128],
                            pt[:, 0:512],
                            start=(i == 0), stop=(i == TK - 1))
                        nc.tensor.matmul(
                            accB[:], vextB[:, i * 128:(i + 1) * 128],
                            pt[:, 512:1024],
                            start=(i == 0), stop=(i == TK - 1))
                    # normalization: l rows sit on the opposite partition
                    # half from O rows; a DMA shifts them into matching
                    # lanes for the DVE reciprocal+mul.
                    lsb = normp.tile([128, 512], F32, tag="lsb")
                    nc.vector.tensor_copy(lsb[64:128, :], accA[64:128, :])
                    nc.vector.tensor_copy(lsb[0:64, :], accB[0:64, :])
                    ldup = normp.tile([128, 512], F32, tag="ldup")
                    nc.sync.dma_start(ldup[0:64, :], lsb[64:128, :])
                    nc.sync.dma_start(ldup[64:128, :], lsb[0:64, :])
                    rbc = normp.tile([128, 512], F32, tag="rbc")
                    nc.vector.reciprocal(rbc[:], ldup[:])
                    nc.vector.tensor_mul(
                        onorm[0:64, q0:q0 + 512], accA[0:64, :], rbc[0:64, :])
                    nc.vector.tensor_mul(
                        onorm[64:128, q0:q0 + 512], accB[64:128, :], rbc[64:128, :])

            # y[qt2*128:.., e] = sum_p onorm_chunk_p^T @ wo_chunk_p
            for qt2 in range(T // 128):
                for e in range(2):
                    ys = yp.tile([128, 512], F32, tag="y")
                    for p in range(2):
                        nc.tensor.matmul(
                            ys[:],
                            onorm[:, p * T + qt2 * 128: p * T + (qt2 + 1) * 128],
                            wo_sb[:, p * 1024 + e * 512: p * 1024 + e * 512 + 512],
                            start=(p == 0), stop=(p == 1))
                    ysb = normp.tile([128, 512], F32, tag="ysb")
                    nc.vector.tensor_copy(ysb[:], ys[:])
                    nc.sync.dma_start(
                        y[qt2 * 128:(qt2 + 1) * 128, e * 512:(e + 1) * 512], ysb[:])


_CACHED = {}


def _get_compiled():
    if "nc" not in _CACHED:
        nc = bacc.Bacc("TRN2", target_bir_lowering=False, debug=False,
                       num_devices=N_CORES, enable_asserts=False)
        with tile.TileContext(nc) as tc:
            build_kernel_body(tc)
        nc.compile()
        _CACHED["nc"] = nc
    return _CACHED["nc"]


def _rot_matrix():
    # rot(x)[0:32] = -x[32:64]; rot(x)[32:64] = x[0:32]
    R = np.zeros((64, 64), np.float32)
    R[np.arange(32), np.arange(32) + 32] = -1.0
    R[np.arange(32) + 32, np.arange(32)] = 1.0
    return R


def _host_inputs(q, k, v, W_q, W_k, W_v, W_o):
    bf = lambda x: np.ascontiguousarray(x).astype(NP_BF16)
    inv_freq = 1.0 / (ROPE_BASE ** (np.arange(0, D_K, 2, dtype=np.float32) / D_K))
    t = np.arange(T, dtype=np.float32)
    freqs = np.outer(t, inv_freq)                      # [T, 32]
    emb = np.concatenate([freqs, freqs], axis=-1)      # [T, 64]
    cosT = np.cos(emb).T                               # [64, T]
    sinT = np.sin(emb).T
    cos2 = bf(np.tile(cosT, (2, 1)))
    sin2 = bf(np.tile(sinT, (2, 1)))
    R = _rot_matrix()
    perm = bf(np.block([[R.T, np.zeros((64, 64), np.float32)],
                        [np.zeros((64, 64), np.float32), R.T]]))

    qT = [bf(q[b].astype(np.float32).T) for b in range(2)]
    kT = [bf(k[b].astype(np.float32).T) for b in range(2)]
    vT = [bf(v[b].astype(np.float32).T) for b in range(2)]

    in_maps = []
    for c in range(N_CORES):
        b, h = divmod(c, N_KV)
        in_maps.append({
            "qT": qT[b],
            "kT": kT[b],
            "vT": vT[b],
            "wq": bf(W_q[:, 256 * h:256 * (h + 1)]),
            "wk2": bf(np.concatenate([W_k[:, 64 * h:64 * (h + 1)]] * 2, axis=1)),
            "wv": bf(W_v[:, 64 * h:64 * (h + 1)]),
            "wo": bf(W_o[256 * h:256 * (h + 1), :]),
            "cos2": cos2,
            "sin2": sin2,
            "perm": perm,
        })
    return in_maps


def run_on_device(q, k, v, W_q, W_k, W_v, W_o, trace=False, **spmd_kwargs):
    nc = _get_compiled()
    in_maps = _host_inputs(q, k, v, W_q, W_k, W_v, W_o)
    res = run_bass_kernel_spmd(nc, in_maps, core_ids=list(range(N_CORES)),
                               trace=trace, **spmd_kwargs)
    partials = [res.results[c]["y"] for c in range(N_CORES)]
    out = np.stack([
        sum(partials[b * N_KV + h] for h in range(N_KV)) for b in range(2)
    ]).astype(np.float32)
    return out, res


def kernel(q, k, v, W_q, W_k, W_v, W_o):
    out, _ = run_on_device(np.asarray(q), np.asarray(k), np.asarray(v),
                           np.asarray(W_q), np.asarray(W_k),
                           np.asarray(W_v), np.asarray(W_o))
    return out


# revision 6
# speedup vs baseline: 1.1887x; 1.1887x over previous
"""GQA attention + RoPE kernel for Trainium2, sharded over 8 NeuronCores.

Sharding: core c handles (batch b = c // 4, kv-head h = c % 4) — one GQA
group (4 query heads + its kv head) per core.  Each core computes its
group's attention end-to-end plus the partial W_o product for its 256
input features; the host sums the 4 partials per batch (the "all-reduce
after W_o" step) and stacks batches.

Device-side layout is fully transposed ("T layout", feature dim on SBUF
partitions, sequence on the free dim):
  - Q^T/K^T projections feed the scores matmul directly (S^T = K @ Q^T),
    so softmax'd probabilities come out already transposed for the P@V
    matmul — no on-chip transposes of the [T, T] score matrix.
  - RoPE is applied in T layout using a sign-permutation matmul (PERM)
    plus elementwise muls with precomputed cos/sin tables.
  - The P@V matmul's stationary operand is [V | ones] (or [ones | V]),
    so each accumulation emits the attention output AND the softmax
    denominator broadcast across 64 partitions in one pass.
  - exp() is evaluated without max-subtraction: scores for this problem
    are bounded (|s| < 4), far from fp32/bf16 overflow.
Numerics: bf16 matmul operands with fp32 PSUM accumulation everywhere;
fp32 partial outputs.

Tiles are split per chunk so Tile's dependency tracking overlaps input
DMA, projections, attention, and the W_o tail; PE work inside the key
sweep is software-pipelined (scores of chunk i+1 are emitted before
P@V of chunk i) so the in-order tensor engine never stalls on exp.
"""

import numpy as np
import ml_dtypes

import concourse.bass as bass
import concourse.tile as tile
from concourse import bacc, mybir
from concourse.bass_utils import run_bass_kernel_spmd

BF16 = mybir.dt.bfloat16
F32 = mybir.dt.float32
NP_BF16 = ml_dtypes.bfloat16

D_MODEL = 1024
T = 2048
N_KV = 4
D_K = 64
ROPE_BASE = 10000.0
N_CORES = 8

DC = D_MODEL // 128  # 8 contraction chunks for projections
TK = T // 128        # 16 key chunks
NQ = T // 512        # 4 query tiles of 512


def build_kernel_body(tc):
    nc = tc.nc

    qT = nc.dram_tensor("qT", [D_MODEL, T], BF16, kind="ExternalInput").ap()
    kT = nc.dram_tensor("kT", [D_MODEL, T], BF16, kind="ExternalInput").ap()
    vT = nc.dram_tensor("vT", [D_MODEL, T], BF16, kind="ExternalInput").ap()
    wq = nc.dram_tensor("wq", [D_MODEL, 256], BF16, kind="ExternalInput").ap()
    wk2 = nc.dram_tensor("wk2", [D_MODEL, 128], BF16, kind="ExternalInput").ap()
    wv = nc.dram_tensor("wv", [D_MODEL, 64], BF16, kind="ExternalInput").ap()
    wo = nc.dram_tensor("wo", [256, 1024], BF16, kind="ExternalInput").ap()
    cos2 = nc.dram_tensor("cos2", [128, T], BF16, kind="ExternalInput").ap()
    sin2 = nc.dram_tensor("sin2", [128, T], BF16, kind="ExternalInput").ap()
    perm = nc.dram_tensor("perm", [128, 128], BF16, kind="ExternalInput").ap()
    y = nc.dram_tensor("y", [T, 1024], F32, kind="ExternalOutput").ap()

    with (
        tc.tile_pool(name="const", bufs=1) as const,
        tc.tile_pool(name="tmp", bufs=3) as tmp,
        tc.tile_pool(name="ptp", bufs=3) as ptp,
        tc.tile_pool(name="norm", bufs=3) as normp,
    ):
        # per-chunk input tiles so consumers start as soon as their chunk lands
        kT_sb = [const.tile([128, T], BF16, tag=f"kT{c}", name=f"kT{c}") for c in range(DC)]
        vT_sb = [const.tile([128, T], BF16, tag=f"vT{c}", name=f"vT{c}") for c in range(DC)]
        qT_sb = [const.tile([128, T], BF16, tag=f"qT{c}", name=f"qT{c}") for c in range(DC)]
        wq_sb = const.tile([128, DC * 256], BF16, tag="wq_sb")
        wk2_sb = const.tile([128, DC * 128], BF16, tag="wk2_sb")
        wv_sb = const.tile([128, DC * 64], BF16, tag="wv_sb")
        wo_sb = const.tile([128, 2 * 1024], BF16, tag="wo_sb")
        cos_sb = const.tile([128, T], BF16, tag="cos_sb")
        sin_sb = const.tile([128, T], BF16, tag="sin_sb")
        perm_sb = const.tile([128, 128], BF16, tag="perm_sb")
        # RoPE'd projections, one tile per 512-column stripe
        krope = [const.tile([128, 512], BF16, tag=f"krope{st}", name=f"krope{st}") for st in range(NQ)]
        qrope = [[const.tile([128, 512], BF16, tag=f"qrope{p}_{st}", name=f"qrope{p}_{st}")
                  for st in range(NQ)] for p in range(2)]
        # [V | ones] / [ones | V] stationary tiles, one per key chunk.  The
        # ones half makes the P@V matmul emit the softmax denominator
        # broadcast over 64 partitions; the mirrored B-form lands head B's
        # output in partitions 64:128 so normalization stays lane-aligned.
        vextA = [const.tile([128, 128], BF16, tag=f"vA{i}", name=f"vA{i}") for i in range(TK)]
        vextB = [const.tile([128, 128], BF16, tag=f"vB{i}", name=f"vB{i}") for i in range(TK)]
        onorm = [[const.tile([128, 512], BF16, tag=f"on{p}_{st}", name=f"on{p}_{st}")
                  for st in range(NQ)] for p in range(2)]

        kq3 = {id(kT): kT.rearrange("(c p) t -> c p t", p=128),
               id(vT): vT.rearrange("(c p) t -> c p t", p=128),
               id(qT): qT.rearrange("(c p) t -> c p t", p=128)}
        for c in range(DC):
            nc.sync.dma_start(kT_sb[c][:], kq3[id(kT)][c])
            nc.sync.dma_start(vT_sb[c][:], kq3[id(vT)][c])
        for w_sb, w_dram, wcols, nch in ((wk2_sb, wk2, 128, DC), (wv_sb, wv, 64, DC),
                                         (wq_sb, wq, 256, DC), (wo_sb, wo, 1024, 2)):
            w3 = w_dram.rearrange("(c p) n -> c p n", p=128)
            for c in range(nch):
                nc.sync.dma_start(w_sb[:, c * wcols:(c + 1) * wcols], w3[c])
        nc.sync.dma_start(cos_sb[:], cos2[:])
        nc.sync.dma_start(sin_sb[:], sin2[:])
        nc.sync.dma_start(perm_sb[:], perm[:])
        for c in range(DC):
            nc.sync.dma_start(qT_sb[c][:], kq3[id(qT)][c])
        for i in range(TK):
            nc.gpsimd.memset(vextA[i][:], 1.0)
            nc.gpsimd.memset(vextB[i][:], 1.0)

        # ---- projections (K2/V first — attention needs them earliest) ----
        with tc.tile_pool(name="pp", bufs=6, space="PSUM") as pp:
            def project_rope(dst, w_sb, wcols, ch, src_sb, st):
                # dst = rope(W_chunk^T @ srcT stripe st)
                ps = pp.tile([128, 512], F32, tag="pp")
                for c in range(DC):
                    nc.tensor.matmul(
                        ps[:],
                        w_sb[:, c * wcols + ch * 128: c * wcols + ch * 128 + 128],
                        src_sb[c][:, st * 512: st * 512 + 512],
                        start=(c == 0), stop=(c == DC - 1))
                xts = tmp.tile([128, 512], BF16, tag="xts")
                nc.vector.tensor_copy(xts[:], ps[:])
                psr = pp.tile([128, 512], F32, tag="pp")
                nc.tensor.matmul(psr[:], perm_sb[:], xts[:], start=True, stop=True)
                t1 = tmp.tile([128, 512], BF16, tag="t1")
                nc.vector.tensor_mul(t1[:], ps[:], cos_sb[:, st * 512:(st + 1) * 512])
                t2 = tmp.tile([128, 512], BF16, tag="t2")
                nc.vector.tensor_mul(t2[:], psr[:], sin_sb[:, st * 512:(st + 1) * 512])
                nc.vector.tensor_add(dst[:], t1[:], t2[:])

            for st in range(NQ):
                project_rope(krope[st], wk2_sb, 128, 0, kT_sb, st)
            for i in range(TK):
                psv = pp.tile([128, 512], F32, tag="pp")
                for c in range(DC):
                    nc.tensor.matmul(
                        psv[:, 0:64],
                        vT_sb[c][:, i * 128:(i + 1) * 128],
                        wv_sb[:, c * 64:(c + 1) * 64],
                        start=(c == 0), stop=(c == DC - 1))
                nc.vector.tensor_copy(vextA[i][:, 0:64], psv[:, 0:64])
                nc.vector.tensor_copy(vextB[i][:, 64:128], psv[:, 0:64])
            for p in range(2):
                for st in range(NQ):
                    project_rope(qrope[p][st], wq_sb, 256, p, qT_sb, st)

        # ---- attention + output projection ----
        with (
            tc.tile_pool(name="stp", bufs=2, space="PSUM") as stp,
            tc.tile_pool(name="accp", bufs=4, space="PSUM") as accp,
        ):
            def scores(p, qt, i):
                # S^T for both heads of the pair: row-tiled concurrent matmuls
                st_ps = stp.tile([128, 1024], F32, tag="st")
                nc.tensor.matmul(
                    st_ps[:, 0:512],
                    krope[i // 4][0:64, (i % 4) * 128:(i % 4 + 1) * 128],
                    qrope[p][qt][0:64, :],
                    start=True, stop=True, tile_position=(0, 0))
                nc.tensor.matmul(
                    st_ps[:, 512:1024],
                    krope[i // 4][64:128, (i % 4) * 128:(i % 4 + 1) * 128],
                    qrope[p][qt][64:128, :],
                    start=True, stop=True, tile_position=(64, 0))
                pt = ptp.tile([128, 1024], BF16, tag="pt")
                nc.scalar.activation(
                    pt[:], st_ps[:], mybir.ActivationFunctionType.Exp, scale=0.125)
                return pt

            def pv(accA, accB, pt, i):
                nc.tensor.matmul(accA[:], vextA[i][:], pt[:, 0:512],
                                 start=(i == 0), stop=(i == TK - 1))
                nc.tensor.matmul(accB[:], vextB[i][:], pt[:, 512:1024],
                                 start=(i == 0), stop=(i == TK - 1))

            for p in range(2):       # head pair (heads 2p, 2p+1)
                for qt in range(NQ):  # 512 queries per sweep
                    accA = accp.tile([128, 512], F32, tag="acc")
                    accB = accp.tile([128, 512], F32, tag="acc")
                    pt_q = [scores(p, qt, 0)]
                    for i in range(TK):  # software pipeline: scores lead PV
                        if i + 1 < TK:
                            pt_q.append(scores(p, qt, i + 1))
                        pv(accA, accB, pt_q[i], i)
                    # move O and l out of PSUM immediately so the acc slots
                    # free up; normalization then runs entirely from SBUF.
                    oab = normp.tile([128, 512], F32, tag="oab")
                    nc.vector.tensor_copy(oab[0:64, :], accA[0:64, :])
                    nc.vector.tensor_copy(oab[64:128, :], accB[64:128, :])
                    lsb = normp.tile([128, 512], F32, tag="lsb")
                    nc.vector.tensor_copy(lsb[64:128, :], accA[64:128, :])
                    nc.vector.tensor_copy(lsb[0:64, :], accB[0:64, :])
                    # l rows sit on the opposite partition half from O rows;
                    # an SBUF->SBUF DMA shifts them into matching lanes.
                    ldup = normp.tile([128, 512], F32, tag="ldup")
                    nc.sync.dma_start(ldup[0:64, :], lsb[64:128, :])
                    nc.sync.dma_start(ldup[64:128, :], lsb[0:64, :])
                    rbc = normp.tile([128, 512], F32, tag="rbc")
                    scr = normp.tile([128, 512], F32, tag="scr")
                    nc.vector.reciprocal_approx_accurate(rbc[:], ldup[:], scr[:])
                    nc.vector.tensor_mul(onorm[p][qt][:], oab[:], rbc[:])

            # y[qt2*128:.., e] = sum_p onorm_chunk_p^T @ wo_chunk_p
            for qt2 in range(T // 128):
                for e in range(2):
                    ys = accp.tile([128, 512], F32, tag="acc")
                    for p in range(2):
                        nc.tensor.matmul(
                            ys[:],
                            onorm[p][qt2 // 4][:, (qt2 % 4) * 128:(qt2 % 4 + 1) * 128],
                            wo_sb[:, p * 1024 + e * 512: p * 1024 + e * 512 + 512],
                            start=(p == 0), stop=(p == 1))
                    ysb = normp.tile([128, 512], F32, tag="ysb")
                    nc.vector.tensor_copy(ysb[:], ys[:])
                    nc.sync.dma_start(
                        y[qt2 * 128:(qt2 + 1) * 128, e * 512:(e + 1) * 512], ysb[:])


_CACHED = {}


def _get_compiled():
    if "nc" not in _CACHED:
        nc = bacc.Bacc("TRN2", target_bir_lowering=False, debug=False,
                       num_devices=N_CORES, enable_asserts=False)
        with tile.TileContext(nc) as tc:
            build_kernel_body(tc)
        nc.compile()
        _CACHED["nc"] = nc
    return _CACHED["nc"]


def _rot_matrix():
    # rot(x)[0:32] = -x[32:64]; rot(x)[32:64] = x[0:32]
    R = np.zeros((64, 64), np.float32)
    R[np.arange(32), np.arange(32) + 32] = -1.0
    R[np.arange(32) + 32, np.arange(32)] = 1.0
    return R


def _host_inputs(q, k, v, W_q, W_k, W_v, W_o):
    bf = lambda x: np.ascontiguousarray(x).astype(NP_BF16)
    inv_freq = 1.0 / (ROPE_BASE ** (np.arange(0, D_K, 2, dtype=np.float32) / D_K))
    t = np.arange(T, dtype=np.float32)
    freqs = np.outer(t, inv_freq)                      # [T, 32]
    emb = np.concatenate([freqs, freqs], axis=-1)      # [T, 64]
    cosT = np.cos(emb).T                               # [64, T]
    sinT = np.sin(emb).T
    cos2 = bf(np.tile(cosT, (2, 1)))
    sin2 = bf(np.tile(sinT, (2, 1)))
    R = _rot_matrix()
    perm = bf(np.block([[R.T, np.zeros((64, 64), np.float32)],
                        [np.zeros((64, 64), np.float32), R.T]]))

    qT = [bf(q[b].astype(np.float32).T) for b in range(2)]
    kT = [bf(k[b].astype(np.float32).T) for b in range(2)]
    vT = [bf(v[b].astype(np.float32).T) for b in range(2)]

    in_maps = []
    for c in range(N_CORES):
        b, h = divmod(c, N_KV)
        in_maps.append({
            "qT": qT[b],
            "kT": kT[b],
            "vT": vT[b],
            "wq": bf(W_q[:, 256 * h:256 * (h + 1)]),
            "wk2": bf(np.concatenate([W_k[:, 64 * h:64 * (h + 1)]] * 2, axis=1)),
            "wv": bf(W_v[:, 64 * h:64 * (h + 1)]),
            "wo": bf(W_o[256 * h:256 * (h + 1), :]),
            "cos2": cos2,
            "sin2": sin2,
            "perm": perm,
        })
    return in_maps


def run_on_device(q, k, v, W_q, W_k, W_v, W_o, trace=False, **spmd_kwargs):
    nc = _get_compiled()
    in_maps = _host_inputs(q, k, v, W_q, W_k, W_v, W_o)
    res = run_bass_kernel_spmd(nc, in_maps, core_ids=list(range(N_CORES)),
                               trace=trace, **spmd_kwargs)
    partials = [res.results[c]["y"] for c in range(N_CORES)]
    out = np.stack([
        sum(partials[b * N_KV + h] for h in range(N_KV)) for b in range(2)
    ]).astype(np.float32)
    return out, res


def kernel(q, k, v, W_q, W_k, W_v, W_o):
    out, _ = run_on_device(np.asarray(q), np.asarray(k), np.asarray(v),
                           np.asarray(W_q), np.asarray(W_k),
                           np.asarray(W_v), np.asarray(W_o))
    return out
